# revision 5
# baseline (speedup 1.0000x reference)
"""Trainium2 Bass kernel for nn_ReallocationMapEncoder.

The reference network is three NAC layers (y = x @ (tanh(W_hat)*sigmoid(M_hat)).T)
applied to a [nsteps, nsyms, nsyms, 3] grid of normalized (t, a, b) indices,
plus a gb broadcast on the trailing axis. NAC is linear in x, so the whole
network collapses to one effective matrix Weff = W3 @ W2 @ W1 of shape [2, 3]:

    y[t, a, b, c] = gb[c] + (t/2)*Weff[c,0] + (a/2048)*Weff[c,1] + (b/2048)*Weff[c,2]

The output [2, 2048, 2048, 2] (67 MB as f32) is a separable affine ramp; the
kernel is purely output-write-bandwidth bound (memory regime).

Device strategy (8 cores, data-parallel on the `a` axis, 256 rows each):
  * emit bf16 on device (the 2e-2 rel-err budget dwarfs bf16's 2^-9 rounding,
    and bf16 keeps f32's exponent range so near-zero outputs stay accurate);
    host upcasts to f32. Halves HBM write traffic vs f32: 4.2 MB/core.
  * the b-index ramp J is a precomputed fp16 input (ints <= 2047 are exact in
    fp16); its load DMA and the tiny bias load are HOISTED into the NEFF
    entry block, before the all-engine start barrier, so the input data+
    completion latency (~4 us) hides entirely under the fixed ~7 us NEFF
    startup (engine boot skew + barrier + register loads).
  * 16 store units of [128, 1024] bf16 are produced by DVE tensor_scalar
    (12 units) and ACT activation(Identity, bias=per-partition AP) (4 units,
    ACT is ~1.8x slower per element):
    out[p, b, c] = J[b]*(Weff[c,2]/nsyms) + bias[p, (t,blk,c)]
  * all DMAs ride the two HWDGE rings: sync (SP) carries the input loads and
    DVE-produced units (one sem wait each -- within the HWDGE 1-wait budget);
    ACT issues its own units' DMAs in program order (zero waits). No SWDGE:
    the Q7 descriptor-generation path cost up to 8.3 us per tile.

Sync-wait slot limits in walrus codegen (HWDGE DMA: 1, DVE/ACT: 2) shape the
structure: per-engine observer copies serialize the two input-DMA sems into
each compute engine's vector clock one at a time, after which compute ops and
ACT's own DMAs need no waits at all.
"""

import numpy as np

NSTEPS = 2
NSYMS = 2048
NCORES = 8
A_PER_CORE = NSYMS // NCORES          # 256
BLKS = A_PER_CORE // 128              # 2 partition blocks per core
F = NSYMS * 2                         # 4096 free elements per a-row (b,c interleaved)
QUARTS = 4                            # column-split of each [128, 4096] row block
UF = F // QUARTS                      # 1024 free elements per store unit
HOIST_INPUT_DMAS = True
JTAB_DTYPE = "float16"

_CACHE = {}


def _build_bass(scales):
    import concourse.bass as bass
    import concourse.mybir as mybir
    from concourse.tile import TileContext

    f32 = mybir.dt.float32
    bf16 = mybir.dt.bfloat16
    jdt = getattr(mybir.dt, JTAB_DTYPE)
    nc = bass.Bass(trn_type="TRN2")

    bias_in = nc.dram_tensor("bias_in", [128, NSTEPS * BLKS * 2], f32, kind="ExternalInput")
    jtab_in = nc.dram_tensor("jtab_in", [128, NSYMS], jdt, kind="ExternalInput")
    out = nc.dram_tensor("out", [NSTEPS, BLKS, 128, F], bf16, kind="ExternalOutput")

    hoist_names = []
    with TileContext(nc) as tc:
        with (
            tc.tile_pool(name="const", bufs=1) as const,
            tc.tile_pool(name="outp", bufs=16) as outp,
        ):
            bias_sb = const.tile([128, NSTEPS * BLKS * 2], f32)
            jtab_sb = const.tile([128, NSYMS], jdt)
            d1 = nc.sync.dma_start(bias_sb[:], bias_in[:])
            d2 = nc.sync.dma_start(jtab_sb[:], jtab_in[:])
            hoist_names = [d1.ins.name, d2.ins.name]

            # Observer copies: fold each input-DMA completion sem into the
            # engine's vector clock with a single-wait instruction, so the
            # real compute ops below need no waits.
            vscr = const.tile([1, 2], f32)
            sscr = const.tile([1, 2], f32)
            nc.vector.tensor_copy(vscr[:, 0:1], bias_sb[0:1, 0:1])
            nc.vector.tensor_copy(vscr[:, 1:2], jtab_sb[0:1, 0:1])
            nc.scalar.copy(sscr[:, 0:1], bias_sb[0:1, 0:1])
            nc.scalar.copy(sscr[:, 1:2], jtab_sb[0:1, 0:1])

            for u in range(NSTEPS * BLKS * QUARTS):
                tb, q = divmod(u, QUARTS)
                t, blk = divmod(tb, BLKS)
                use_act = (u % 4 == 3)
                ot = outp.tile([128, UF], bf16)
                otv = ot[:].rearrange("p (b c) -> p b c", c=2)
                jsl = jtab_sb[:, q * (UF // 2) : (q + 1) * (UF // 2)]
                for c in range(2):
                    idx = (t * BLKS + blk) * 2 + c
                    bap = bias_sb[:, idx : idx + 1]
                    if use_act:
                        nc.scalar.activation(
                            otv[:, :, c], jsl,
                            mybir.ActivationFunctionType.Identity,
                            bias=bap, scale=scales[c],
                        )
                    else:
                        nc.vector.tensor_scalar(
                            otv[:, :, c], jsl, scales[c], bap,
                            mybir.AluOpType.mult, mybir.AluOpType.add,
                        )
                dst = out[t, blk][:, q * UF : (q + 1) * UF]
                if use_act:
                    nc.scalar.dma_start(dst, ot[:])
                else:
                    nc.sync.dma_start(dst, ot[:])

    if HOIST_INPUT_DMAS:
        _hoist_input_dmas(nc, mybir, hoist_names)
    _legalize_waits(nc, mybir)
    return nc


def _hoist_input_dmas(nc, mybir, names):
    """Move the (dependency-free) input-load DMAs from the tile block into
    the NEFF entry block, ahead of SP's drain + all-engine start barrier.
    Their data transfer + completion receipt then overlaps the fixed NEFF
    startup costs instead of following them. Sems only fire EARLIER, so all
    downstream waits stay correct."""
    func = nc.m.functions[0]
    entry = func.blocks[0]
    moved = []
    for block in func.blocks[1:]:
        keep = []
        for inst in block.instructions:
            if inst.name in names:
                moved.append(inst)
            else:
                keep.append(inst)
        if len(keep) != len(block.instructions):
            block.instructions = keep
    assert len(moved) == len(names), (len(moved), names)
    moved.sort(key=lambda i: names.index(i.name))
    # insert before SP's first Drain (which precedes SP's barrier join)
    insts = list(entry.instructions)
    pos = len(insts)
    for k, inst in enumerate(insts):
        if inst.engine == mybir.EngineType.SP and isinstance(inst, mybir.InstDrain):
            pos = k
            break
    entry.instructions = insts[:pos] + moved + insts[pos:]


def _legalize_waits(nc, mybir):
    """This walrus build fits very few semaphore waits per instruction (one
    for most engine structs). Tile's auto-generated kernel-tail drain waits
    on every DMA lane + engine sem at once; split any multi-wait instruction
    into a chain of single-wait Drain carriers on the same engine."""
    for func in nc.m.functions:
        for block in func.blocks:
            insts = list(block.instructions)
            new_insts = []
            changed = False
            for inst in insts:
                si = inst.sync_info
                waits = list(si.on_wait) if si is not None and si.on_wait else []
                if len(waits) > 1:
                    for w in waits[:-1]:
                        d = mybir.InstDrain(
                            name=f"{inst.name}-waitsplit-{len(new_insts)}",
                            ins=[],
                            outs=[],
                            bass_is_fusable=False,
                        )
                        d.engine = inst.engine
                        d.sync_info = mybir.SyncInfo(on_wait=[w], on_update=[])
                        new_insts.append(d)
                    inst.sync_info = mybir.SyncInfo(
                        on_wait=[waits[-1]], on_update=list(si.on_update or [])
                    )
                    changed = True
                new_insts.append(inst)
            if changed:
                block.instructions = new_insts


def _host_consts(gb, w_hat1, m_hat1, w_hat2, m_hat2, w_hat3, m_hat3):
    def nacw(w, m):
        w = np.asarray(w, np.float64)
        m = np.asarray(m, np.float64)
        return np.tanh(w) * (1.0 / (1.0 + np.exp(-m)))

    weff = nacw(w_hat3, m_hat3) @ nacw(w_hat2, m_hat2) @ nacw(w_hat1, m_hat1)  # [2,3]
    gb = np.asarray(gb, np.float64)

    scales = [float(np.float32(weff[c, 2] / NSYMS)) for c in range(2)]

    # bias[core][p, (t,blk,c)] = gb[c] + (t/2)Weff[c,0] + (a/2048)Weff[c,1]
    biases = []
    for core in range(NCORES):
        bias = np.empty((128, NSTEPS, BLKS, 2), np.float64)
        for t in range(NSTEPS):
            for blk in range(BLKS):
                a = (core * A_PER_CORE + blk * 128 + np.arange(128)) / NSYMS
                for c in range(2):
                    bias[:, t, blk, c] = (
                        gb[c] + (t / NSTEPS) * weff[c, 0] + a * weff[c, 1]
                    )
        biases.append(np.ascontiguousarray(bias.reshape(128, -1), np.float32))

    jtab = np.ascontiguousarray(
        np.broadcast_to(np.arange(NSYMS, dtype=np.float32), (128, NSYMS)).astype(
            JTAB_DTYPE
        )
    )
    return scales, biases, jtab


def kernel(market, gb, w_hat1, m_hat1, w_hat2, m_hat2, w_hat3, m_hat3):
    from concourse.bass_utils import run_bass_kernel_spmd

    scales, biases, jtab = _host_consts(
        gb, w_hat1, m_hat1, w_hat2, m_hat2, w_hat3, m_hat3
    )
    # the tensor_scalar immediates (scales) are baked into the traced program,
    # so the compiled module is keyed on them
    key = ("nc", tuple(scales))
    if key not in _CACHE:
        _CACHE[key] = _build_bass(scales)
    nc = _CACHE[key]
    _CACHE["last_nc"] = nc

    in_maps = [
        {"bias_in": biases[core], "jtab_in": jtab} for core in range(NCORES)
    ]
    res = run_bass_kernel_spmd(nc, in_maps, core_ids=list(range(NCORES)))
    parts = [
        np.asarray(r["out"]).reshape(NSTEPS, A_PER_CORE, NSYMS, 2).astype(np.float32)
        for r in res.results
    ]
    return np.concatenate(parts, axis=1)


# revision 6
# speedup vs baseline: 1.2020x; 1.2020x over previous
"""Trainium2 Bass kernel for nn_ReallocationMapEncoder.

The reference network is three NAC layers (y = x @ (tanh(W_hat)*sigmoid(M_hat)).T)
applied to a [nsteps, nsyms, nsyms, 3] grid of normalized (t, a, b) indices,
plus a gb broadcast on the trailing axis. NAC is linear in x, so the whole
network collapses to one effective matrix Weff = W3 @ W2 @ W1 of shape [2, 3]:

    y[t, a, b, c] = gb[c] + (t/2)*Weff[c,0] + (a/2048)*Weff[c,1] + (b/2048)*Weff[c,2]

The output [2, 2048, 2048, 2] (67 MB as f32) is a separable affine ramp; the
kernel is purely output-write-bandwidth bound (memory regime).

Device strategy (8 cores, data-parallel on the `a` axis, 256 rows each):
  * emit bf16 on device (the 2e-2 rel-err budget dwarfs bf16's 2^-9 rounding,
    and bf16 keeps f32's exponent range so near-zero outputs stay accurate);
    host upcasts to f32. Halves HBM write traffic vs f32: 4.2 MB/core.
  * the b-index ramp J is generated by a gpsimd iota (f32, exact for ints
    <= 2047) -- it finishes (~5.9us) before the bias input-DMA completion
    (~6.3us, the real gate: any first DMA has ~4us fixed latency on this
    runtime), so the only external input is the tiny bias table.
  * store units are produced by two engines in parallel and written out on
    the two HWDGE rings (no SWDGE -- Q7 descriptor-gen cost up to 8.3us):
      - DVE tensor_scalar, 12 units of [128, 1024] bf16, DMAs issued by the
        otherwise-idle sync (SP) engine with a single producer-sem wait;
      - ACT activation(Identity, bias=per-partition AP), 2 units of
        [128, 2048] (ACT is ~2x slower/elem), DMAs issued by ACT itself in
        program order (zero waits);
    out[p, b, c] = J[b]*(Weff[c,2]/nsyms) + bias[p, (t,blk,c)]
  * the Tile entry all-engine barrier is stripped post-build: every cross-
    engine dependency is carried by monotonic >= sem waits that the runtime
    zeroes per execution, and the barrier otherwise gates the whole kernel
    on the Tensor engine's ~3.2us sequencer boot that nothing here uses.
  * DMAHW lane-recycle waits on output DMAs are dropped post-build: HWDGE
    descriptors on one ring complete in FIFO order, and no instruction waits
    an intermediate value of a recycled output lane -- only the kernel-tail
    drain waits the final counts. Keeping them forced an extra single-wait
    Drain carrier (~0.5us) per DMA on the issuing engine (walrus fits one
    sem wait per HWDGE DMA).
"""

import numpy as np

NSTEPS = 2
NSYMS = 2048
NCORES = 8
A_PER_CORE = NSYMS // NCORES          # 256
BLKS = A_PER_CORE // 128              # 2 partition blocks per core
F = NSYMS * 2                         # 4096 free elements per a-row (b,c interleaved)
DU = 1024                             # DVE store-unit columns
AU = 2048                             # ACT store-unit columns
STRIP_ENTRY_BARRIER = True
DROP_RECYCLE_WAITS = True

_CACHE = {}

# (t, blk, colstart, ncols, engine) store schedule: ACT takes the tail half
# of the (t, 1) row-blocks; DVE covers the rest in 1024-col units.
_UNITS = []
for _t in range(NSTEPS):
    for _blk in range(BLKS):
        if _blk == BLKS - 1:
            for _q in range(2):
                _UNITS.append((_t, _blk, _q * DU, DU, "dve"))
            _UNITS.append((_t, _blk, 2 * DU, AU, "act"))
        else:
            for _q in range(4):
                _UNITS.append((_t, _blk, _q * DU, DU, "dve"))


def _build_bass(scales):
    import concourse.bass as bass
    import concourse.mybir as mybir
    from concourse.tile import TileContext

    f32 = mybir.dt.float32
    bf16 = mybir.dt.bfloat16
    nc = bass.Bass(trn_type="TRN2")

    bias_in = nc.dram_tensor("bias_in", [128, NSTEPS * BLKS * 2], f32, kind="ExternalInput")
    out = nc.dram_tensor("out", [NSTEPS, BLKS, 128, F], bf16, kind="ExternalOutput")

    with TileContext(nc) as tc:
        with (
            tc.tile_pool(name="const", bufs=1) as const,
            tc.tile_pool(name="outp", bufs=len(_UNITS)) as outp,
        ):
            bias_sb = const.tile([128, NSTEPS * BLKS * 2], f32)
            d1 = nc.sync.dma_start(bias_sb[:], bias_in[:])
            hoist_names = [d1.ins.name]

            jtab_sb = const.tile([128, NSYMS], f32)
            nc.gpsimd.iota(
                jtab_sb[:], pattern=[[1, NSYMS]], base=0, channel_multiplier=0,
                allow_small_or_imprecise_dtypes=True,
            )

            # Observer copies: fold the bias-DMA lane sem and the gpsimd
            # (Pool) iota sem into each compute engine's vector clock with
            # single-wait instructions, so the real compute ops below and
            # ACT's own DMAs need no waits at all.
            vscr = const.tile([1, 2], f32)
            sscr = const.tile([1, 2], f32)
            nc.vector.tensor_copy(vscr[:, 0:1], bias_sb[0:1, 0:1])
            nc.vector.tensor_copy(vscr[:, 1:2], jtab_sb[0:1, 0:1])
            nc.scalar.copy(sscr[:, 0:1], bias_sb[0:1, 0:1])
            nc.scalar.copy(sscr[:, 1:2], jtab_sb[0:1, 0:1])

            for t, blk, col0, ncols, eng in _UNITS:
                ot = outp.tile([128, ncols], bf16)
                otv = ot[:].rearrange("p (b c) -> p b c", c=2)
                jsl = jtab_sb[:, col0 // 2 : col0 // 2 + ncols // 2]
                for c in range(2):
                    idx = (t * BLKS + blk) * 2 + c
                    bap = bias_sb[:, idx : idx + 1]
                    if eng == "act":
                        nc.scalar.activation(
                            otv[:, :, c], jsl,
                            mybir.ActivationFunctionType.Identity,
                            bias=bap, scale=scales[c],
                        )
                    else:
                        nc.vector.tensor_scalar(
                            otv[:, :, c], jsl, scales[c], bap,
                            mybir.AluOpType.mult, mybir.AluOpType.add,
                        )
                dst = out[t, blk][:, col0 : col0 + ncols]
                if eng == "act":
                    nc.scalar.dma_start(dst, ot[:])
                else:
                    nc.sync.dma_start(dst, ot[:])

    if STRIP_ENTRY_BARRIER:
        _strip_entry_barrier(nc, mybir)
    _hoist_input_dmas(nc, mybir, hoist_names)
    if DROP_RECYCLE_WAITS:
        _drop_recycle_waits(nc, mybir)
    _legalize_waits(nc, mybir)
    return nc


def _strip_entry_barrier(nc, mybir):
    """Remove the all-engine start barrier (both butterfly phases) and its
    paired Drains from the NEFF entry block. All kernel dependencies are
    monotonic >= waits on runtime-zeroed sems, so engines can start their
    streams immediately; the barrier only serialized everyone behind the
    slowest engine's (Tensor, unused here) ~3us sequencer boot. The exit
    barrier is kept."""
    entry = nc.m.functions[0].blocks[0]
    keep = []
    for inst in entry.instructions:
        if isinstance(inst, mybir.InstEventSemaphore) and inst.name.startswith(
            "barrier_"
        ):
            continue
        if isinstance(inst, mybir.InstDrain):
            continue
        keep.append(inst)
    entry.instructions = keep


def _hoist_input_dmas(nc, mybir, names):
    """Move the (dependency-free) input-load DMAs from the tile block into
    the NEFF entry block so they dispatch as early as possible. Sems only
    fire EARLIER, so all downstream waits stay correct."""
    func = nc.m.functions[0]
    entry = func.blocks[0]
    moved = []
    for block in func.blocks[1:]:
        keep = []
        for inst in block.instructions:
            if inst.name in names:
                moved.append(inst)
            else:
                keep.append(inst)
        if len(keep) != len(block.instructions):
            block.instructions = keep
    assert len(moved) == len(names), (len(moved), names)
    moved.sort(key=lambda i: names.index(i.name))
    insts = list(entry.instructions)
    pos = len(insts)
    for k, inst in enumerate(insts):
        if inst.engine == mybir.EngineType.SP and isinstance(
            inst, (mybir.InstDrain, mybir.InstUnconditionalBranch)
        ):
            pos = k
            break
    entry.instructions = insts[:pos] + moved + insts[pos:]


def _drop_recycle_waits(nc, mybir):
    """Output DMAs whose DMAHW completion lane is recycled get a second
    'previous lane user done' wait from Tile. Descriptors on one HWDGE ring
    complete in FIFO order and nothing waits intermediate values of
    recycled output lanes (the kernel-tail drain waits the final counts),
    so the wait is redundant -- and walrus only fits ONE wait per HWDGE
    DMA, forcing a costly extra Drain carrier. Drop DMAHW-sem waits from
    any DMA that also carries a producer-engine wait, and from ACT-issued
    DMAs ordered by program order."""
    func = nc.m.functions[0]
    for block in func.blocks:
        for inst in block.instructions:
            if not isinstance(inst, mybir.InstDMACopy):
                continue
            si = inst.sync_info
            waits = list(si.on_wait) if si is not None and si.on_wait else []
            if not waits:
                continue
            kept = [w for w in waits if not str(getattr(w, "ant_name", "")).startswith("DMAHW")]
            if len(kept) != len(waits):
                inst.sync_info = mybir.SyncInfo(
                    on_wait=kept, on_update=list(si.on_update or [])
                )


def _legalize_waits(nc, mybir):
    """This walrus build fits very few semaphore waits per instruction (one
    for most engine structs). Tile's auto-generated kernel-tail drain waits
    on every DMA lane + engine sem at once; split any multi-wait instruction
    into a chain of single-wait Drain carriers on the same engine."""
    for func in nc.m.functions:
        for block in func.blocks:
            insts = list(block.instructions)
            new_insts = []
            changed = False
            for inst in insts:
                si = inst.sync_info
                waits = list(si.on_wait) if si is not None and si.on_wait else []
                if len(waits) > 1:
                    for w in waits[:-1]:
                        d = mybir.InstDrain(
                            name=f"{inst.name}-waitsplit-{len(new_insts)}",
                            ins=[],
                            outs=[],
                            bass_is_fusable=False,
                        )
                        d.engine = inst.engine
                        d.sync_info = mybir.SyncInfo(on_wait=[w], on_update=[])
                        new_insts.append(d)
                    inst.sync_info = mybir.SyncInfo(
                        on_wait=[waits[-1]], on_update=list(si.on_update or [])
                    )
                    changed = True
                new_insts.append(inst)
            if changed:
                block.instructions = new_insts


def _host_consts(gb, w_hat1, m_hat1, w_hat2, m_hat2, w_hat3, m_hat3):
    def nacw(w, m):
        w = np.asarray(w, np.float64)
        m = np.asarray(m, np.float64)
        return np.tanh(w) * (1.0 / (1.0 + np.exp(-m)))

    weff = nacw(w_hat3, m_hat3) @ nacw(w_hat2, m_hat2) @ nacw(w_hat1, m_hat1)  # [2,3]
    gb = np.asarray(gb, np.float64)

    scales = [float(np.float32(weff[c, 2] / NSYMS)) for c in range(2)]

    # bias[core][p, (t,blk,c)] = gb[c] + (t/2)Weff[c,0] + (a/2048)Weff[c,1]
    biases = []
    for core in range(NCORES):
        bias = np.empty((128, NSTEPS, BLKS, 2), np.float64)
        for t in range(NSTEPS):
            for blk in range(BLKS):
                a = (core * A_PER_CORE + blk * 128 + np.arange(128)) / NSYMS
                for c in range(2):
                    bias[:, t, blk, c] = (
                        gb[c] + (t / NSTEPS) * weff[c, 0] + a * weff[c, 1]
                    )
        biases.append(np.ascontiguousarray(bias.reshape(128, -1), np.float32))
    return scales, biases


def kernel(market, gb, w_hat1, m_hat1, w_hat2, m_hat2, w_hat3, m_hat3):
    from concourse.bass_utils import run_bass_kernel_spmd

    scales, biases = _host_consts(
        gb, w_hat1, m_hat1, w_hat2, m_hat2, w_hat3, m_hat3
    )
    # the tensor_scalar immediates (scales) are baked into the traced program,
    # so the compiled module is keyed on them
    key = ("nc", tuple(scales))
    if key not in _CACHE:
        _CACHE[key] = _build_bass(scales)
    nc = _CACHE[key]
    _CACHE["last_nc"] = nc

    in_maps = [{"bias_in": biases[core]} for core in range(NCORES)]
    res = run_bass_kernel_spmd(nc, in_maps, core_ids=list(range(NCORES)))
    parts = [
        np.asarray(r["out"]).reshape(NSTEPS, A_PER_CORE, NSYMS, 2).astype(np.float32)
        for r in res.results
    ]
    return np.concatenate(parts, axis=1)
